# revision 1
# baseline (speedup 1.0000x reference)
"""Multi-head causal attention on 8 Trainium2 NeuronCores.

Sharding: core c -> batch b = c // 4, head group g = c % 4 (4 of 16 heads).
Each core computes q/k/v for its 4 heads, causal softmax attention, and a
partial output  z_norm @ W_O[heads]  of shape [S, D].  Host sums the 4
head-group partials per batch and adds b_O.

Device kernel (per core, all matmuls in float32r = full-rate fp32):
  Phase A: qT/kT/vT[h] = W.T @ x.T   (+bias, q scaled by 1/sqrt(dh)),
           streamed to a DRAM scratch (SBUF cannot hold x.T + all outputs).
  Phase B: per head: scores = qT.T @ kT (i on partitions, j on free),
           additive causal mask on the diagonal 512-chunk, exp with fused
           row-sum accumulation on ACT, normalization folded into the PE
           transpose (rhs = identity * recip  instead of identity),
           zT += v_tile.T @ pT.
  Phase C: out[s_tile, d_chunk] = sum_h zT_h.T @ Wo_h.
"""

import sys

for _p in ("/opt/trn_rl_repo",):
    if _p not in sys.path:
        sys.path.insert(0, _p)

import numpy as np

import concourse.bass as bass
from concourse import bacc
import concourse.mybir as mybir
import concourse.tile as tile
from concourse.bass_utils import run_bass_kernel_spmd
from concourse.masks import make_identity

F32 = mybir.dt.float32
F32R = mybir.dt.float32r
BF16 = mybir.dt.float16  # fp16: same matmul rate, 4x finer mantissa than bf16

B, S, D, H, E = 2, 2048, 2048, 16, 128
HL = 4          # heads per core
NCORES = 8
P = 128         # partitions
CH = 512        # free-dim chunk
S_T = S // P    # 16 seq tiles
S_C = S // CH   # 4 seq chunks
D_T = D // P    # 16 model-dim subtiles
D_C = D // CH   # 4 model-dim chunks
INV_SQRT_E = 1.0 / float(np.sqrt(E))


def r(ap):
    return ap.bitcast(F32R)


def _trace_kernel(tc, xt, wq, wk, wv, wo, bq, bk, bv, outp):
    nc = tc.nc
    ts = bass.ts

    xt3 = xt.rearrange("(o p) s -> p o s", p=P)            # [128, 16, 2048]
    w3 = [w.rearrange("(o p) e -> p o e", p=P) for w in (wq, wk, wv)]
    wo3 = wo.rearrange("(h p) d -> p h d", p=P)            # [128, 4, 2048]
    out3 = outp.rearrange("(t p) d -> t p d", p=P)         # [16, 128, 2048]

    from contextlib import ExitStack

    with ExitStack() as top:
        const_pool = top.enter_context(tc.tile_pool(name="consts", bufs=1))
        dram = top.enter_context(tc.tile_pool(name="dram", bufs=1, space="DRAM"))

        # qT/kT/vT scratch: [12, 128, 2048] (m*4 + h)
        qkvT = dram.tile([3 * HL, P, S], F32R)

        identity_f = const_pool.tile([P, P], F32)
        make_identity(nc, identity_f)
        identity = const_pool.tile([P, P], F32R)
        nc.vector.tensor_copy(identity, identity_f)

        # additive causal masks for the diagonal 512-chunk, one per (i % 4)
        zpool = top.enter_context(tc.tile_pool(name="zT", bufs=1))
        zT = zpool.tile([P, HL, S], F32R)  # persists into phase C

        # transposed causal triangle for the diagonal 128-block of scoresT:
        # valid iff local col >= p  (j <= i)
        dmask = const_pool.tile([P, P], F32)
        nc.gpsimd.memset(dmask, 0.0)
        nc.gpsimd.affine_select(
            out=dmask,
            in_=dmask,
            compare_op=mybir.AluOpType.is_ge,
            fill=-1e9,
            base=0,
            pattern=[[1, P]],
            channel_multiplier=-1,
        )

        biases = const_pool.tile([P, 3, HL], F32)
        for m, bsrc in enumerate((bq, bk, bv)):
            nc.gpsimd.dma_start(biases[:, m, :], bsrc.rearrange("(h p) -> p h", p=P))

        # ---------------- Phase A: q/k/v projections ----------------
        with ExitStack() as pa:
            wpool = pa.enter_context(tc.tile_pool(name="wqkv", bufs=1))
            xpool = pa.enter_context(tc.tile_pool(name="xchunk", bufs=2))
            stage = pa.enter_context(tc.tile_pool(name="astage", bufs=3))
            psA = pa.enter_context(tc.tile_pool(name="psA", bufs=7, space="PSUM"))

            w_sb = [
                wpool.tile([P, D_T, HL * E], F32R, name=f"w{m}") for m in range(3)
            ]
            xc0 = xpool.tile([P, D_T, CH], F32R, name="xc")
            # interleave so matmul d=0 operands (xc d0, w0 d0) arrive first
            for d in range(D_T):
                nc.sync.dma_start(xc0[:, d, :], xt3[:, d, ts(0, CH)])
                for m in range(3):
                    nc.sync.dma_start(w_sb[m][:, d, :], w3[m][:, d, :])

            groups = [(m, h) for m in range(3) for h in range(HL)]
            for c in range(S_C):
                if c == 0:
                    xc = xc0
                else:
                    xc = xpool.tile([P, D_T, CH], F32R, name="xc")
                    for d in range(D_T):
                        nc.sync.dma_start(xc[:, d, :], xt3[:, d, ts(c, CH)])
                # d-outer in waves of 6 psum groups: PE consumes weight/x
                # slices at DMA-arrival order instead of stalling on full
                # weight tensors (matters for chunk 0)
                for wave in (groups[:6], groups[6:]):
                    pss = {g: psA.tile([P, CH], F32, name="psA") for g in wave}
                    for d in range(D_T):
                        for (m, h) in wave:
                            nc.tensor.matmul(
                                pss[(m, h)],
                                w_sb[m][:, d, ts(h, E)],
                                xc[:, d, :],
                                start=(d == 0),
                                stop=(d == D_T - 1),
                            )
                    for (m, h) in wave:
                        st = stage.tile([P, CH], F32R, name="st")
                        # q: bq is pre-scaled by 1/sqrt(E) on host, so
                        # (ps + b)/sqrt(E) == ps*scale + b_scaled
                        nc.vector.tensor_scalar(
                            st, pss[(m, h)],
                            INV_SQRT_E if m == 0 else 1.0,
                            biases[:, m, h, None],
                            op0=mybir.AluOpType.mult,
                            op1=mybir.AluOpType.add,
                        )
                        nc.sync.dma_start(qkvT[m * HL + h, :, ts(c, CH)], st)

        # ---------------- Phase B: attention per head ----------------
        # scoresT[j, i] computed directly (kT stationary, qT moving); exp only
        # over the valid column range; PV uses v augmented with a ones column
        # so z_ps[:, 128] holds softmax row-sums on the i partitions; z is
        # normalized per partition, transposed into zT for phase C.
        with ExitStack() as pb:
            qkvp = pb.enter_context(tc.tile_pool(name="qkv", bufs=2))
            vtp = pb.enter_context(tc.tile_pool(name="vtp", bufs=1))
            vnp = pb.enter_context(tc.tile_pool(name="vnat", bufs=2))
            epool = pb.enter_context(tc.tile_pool(name="expT", bufs=3))
            zsp = pb.enter_context(tc.tile_pool(name="zsb", bufs=4))
            small = pb.enter_context(tc.tile_pool(name="small", bufs=4))
            psS = pb.enter_context(tc.tile_pool(name="psS", bufs=4, space="PSUM"))
            psT = pb.enter_context(tc.tile_pool(name="psT", bufs=2, space="PSUM"))
            psZ = pb.enter_context(tc.tile_pool(name="psZ", bufs=2, space="PSUM"))

            for lh in range(HL):
                qT = qkvp.tile([P, S], F32R, name="qT")
                kT = qkvp.tile([P, S], F32R, name="kT")
                vT = vtp.tile([P, S], F32R, name="vT")
                for cc in range(S_C):
                    nc.sync.dma_start(qT[:, ts(cc, CH)], qkvT[0 * HL + lh, :, ts(cc, CH)])
                    nc.sync.dma_start(kT[:, ts(cc, CH)], qkvT[1 * HL + lh, :, ts(cc, CH)])
                    nc.sync.dma_start(vT[:, ts(cc, CH)], qkvT[2 * HL + lh, :, ts(cc, CH)])
                # v natural [j, e] + ones column (fused row-sums), fp16
                v_aug = vnp.tile([P, S_T, E + 1], BF16, name="v_aug")
                nc.vector.memset(v_aug[:, :, E : E + 1], 1.0)
                for jt in range(S_T):
                    tpv = psT.tile([P, P], F32R, name="tp")
                    nc.tensor.transpose(tpv, vT[:, ts(jt, P)], identity)
                    nc.vector.tensor_copy(v_aug[:, jt, :E], tpv)

                for c in range(S_C):     # i-chunks of 512
                    n_jt = S_C * c + 4
                    expT = epool.tile([P, S_T, CH], BF16, name="expT")
                    for jt in range(n_jt):
                        sps = psS.tile([P, CH], F32, name="sps")
                        nc.tensor.matmul(
                            sps,
                            kT[:, ts(jt, P)],
                            qT[:, ts(c, CH)],
                            start=True,
                            stop=True,
                        )
                        b = jt - S_C * c
                        if b >= 0:
                            # mask the 128-wide diagonal block; cols < b*128
                            # are never read by PV, cols beyond are valid
                            nc.vector.tensor_add(
                                sps[:, ts(b, P)], sps[:, ts(b, P)], dmask
                            )
                            nc.scalar.activation(
                                expT[:, jt, b * P :],
                                sps[:, b * P :],
                                mybir.ActivationFunctionType.Exp,
                            )
                        else:
                            nc.scalar.activation(
                                expT[:, jt, :],
                                sps,
                                mybir.ActivationFunctionType.Exp,
                            )
                    for a in range(S_C):  # i-tile within chunk
                        i = S_C * c + a
                        z_ps = psZ.tile([P, E + 1], F32, name="z_ps")
                        for jt in range(i + 1):
                            nc.tensor.matmul(
                                z_ps,
                                expT[:, jt, ts(a, P)],
                                v_aug[:, jt, :],
                                start=(jt == 0),
                                stop=(jt == i),
                            )
                        rec = small.tile([P, 1], F32, name="rec")
                        nc.vector.reciprocal(rec, z_ps[:, E : E + 1])
                        z_sb = zsp.tile([P, E], F32R, name="z_sb")
                        nc.vector.tensor_scalar_mul(z_sb, z_ps[:, :E], rec)
                        tpz = psT.tile([P, P], F32R, name="tp")
                        nc.tensor.transpose(tpz, z_sb, identity)
                        nc.vector.tensor_copy(zT[:, lh, ts(i, P)], tpz)

        # ---------------- Phase C: output projection ----------------
        with ExitStack() as pc:
            wop = pc.enter_context(tc.tile_pool(name="wo", bufs=1))
            ostage = pc.enter_context(tc.tile_pool(name="ostage", bufs=3))
            psC = pc.enter_context(tc.tile_pool(name="psC", bufs=2, space="PSUM"))

            wo_sb = wop.tile([P, HL, D], F32R)
            for lh in range(HL):
                for dc in range(D_C):
                    nc.sync.dma_start(wo_sb[:, lh, ts(dc, CH)], wo3[:, lh, ts(dc, CH)])

            for t in range(S_T):
                for dc in range(D_C):
                    ops = psC.tile([P, CH], F32, name="ops")
                    for lh in range(HL):
                        nc.tensor.matmul(
                            ops,
                            zT[:, lh, ts(t, P)],
                            wo_sb[:, lh, ts(dc, CH)],
                            start=(lh == 0),
                            stop=(lh == HL - 1),
                        )
                    ot = ostage.tile([P, CH], F32, name="ot")
                    nc.vector.tensor_copy(ot, ops)
                    nc.sync.dma_start(out3[t, :, ts(dc, CH)], ot)


_NC_CACHE = {}
LAST_RESULTS = None


def _get_nc():
    if "nc" not in _NC_CACHE:
        nc = bacc.Bacc("TRN2", target_bir_lowering=False, debug=False)
        xt = nc.dram_tensor("xt", [D, S], F32R, kind="ExternalInput")
        wq = nc.dram_tensor("wq", [D, HL * E], F32R, kind="ExternalInput")
        wk = nc.dram_tensor("wk", [D, HL * E], F32R, kind="ExternalInput")
        wv = nc.dram_tensor("wv", [D, HL * E], F32R, kind="ExternalInput")
        wo = nc.dram_tensor("wo", [HL * E, D], F32R, kind="ExternalInput")
        bq = nc.dram_tensor("bq", [HL * E], F32, kind="ExternalInput")
        bk = nc.dram_tensor("bk", [HL * E], F32, kind="ExternalInput")
        bv = nc.dram_tensor("bv", [HL * E], F32, kind="ExternalInput")
        outp = nc.dram_tensor("outp", [S, D], F32, kind="ExternalOutput")
        with tile.TileContext(nc) as tc:
            _trace_kernel(tc, xt, wq, wk, wv, wo, bq, bk, bv, outp)
        nc.compile()
        _NC_CACHE["nc"] = nc
    return _NC_CACHE["nc"]


def kernel(normalized_resid_pre, W_Q, W_K, W_V, W_O, b_Q, b_K, b_V, b_O):
    x = np.asarray(normalized_resid_pre, np.float32)
    W_Q = np.asarray(W_Q, np.float32)
    W_K = np.asarray(W_K, np.float32)
    W_V = np.asarray(W_V, np.float32)
    W_O = np.asarray(W_O, np.float32)
    b_Q = np.asarray(b_Q, np.float32)
    b_K = np.asarray(b_K, np.float32)
    b_V = np.asarray(b_V, np.float32)
    b_O = np.asarray(b_O, np.float32)

    nc = _get_nc()
    in_maps = []
    for core in range(NCORES):
        b, g = core // (NCORES // B), core % (NCORES // B)
        hs = range(g * HL, (g + 1) * HL)
        in_maps.append(
            {
                "xt": np.ascontiguousarray(x[b].T),
                "wq": np.ascontiguousarray(np.concatenate([W_Q[h] for h in hs], 1)),
                "wk": np.ascontiguousarray(np.concatenate([W_K[h] for h in hs], 1)),
                "wv": np.ascontiguousarray(np.concatenate([W_V[h] for h in hs], 1)),
                "wo": np.ascontiguousarray(
                    W_O[g * HL : (g + 1) * HL].reshape(HL * E, D)
                ),
                "bq": np.ascontiguousarray(b_Q[g * HL : (g + 1) * HL].reshape(-1) * np.float32(INV_SQRT_E)),
                "bk": np.ascontiguousarray(b_K[g * HL : (g + 1) * HL].reshape(-1)),
                "bv": np.ascontiguousarray(b_V[g * HL : (g + 1) * HL].reshape(-1)),
            }
        )

    res = run_bass_kernel_spmd(nc, in_maps, core_ids=list(range(NCORES)))
    global LAST_RESULTS
    LAST_RESULTS = res
    out = np.zeros((B, S, D), np.float32)
    for core in range(NCORES):
        out[core // (NCORES // B)] += res.results[core]["outp"]
    out += b_O[None, None, :]
    return out

